# revision 34
# baseline (speedup 1.0000x reference)
"""Trainium2 Bass kernel for MiMoAudio attention (GQA + neox RoPE + causal softmax + o_proj).

Strategy (tensor-parallel over heads, 8 cores):
  - Each core owns 2 of the 16 q heads (128 q channels) and the single kv head
    (64 channels) that those q heads attend to (GQA group).
  - Host pre-transposes hidden_states to xT [H, B*S] so every on-device matmul
    contracts over the partition dim with no on-device transposition of x.
  - All activations live "feature-on-partitions" (transposed domain):
      qT [128, T], kT/vT in kvT [128, T], scoresT [j, i], attnT [d', i].
    Softmax runs without max-subtraction (logits are O(10), fp32-exp safe);
    the denominator is obtained by an appended ones-row in the PV matmul.
  - o_proj row-slice per core produces a partial [T, H] output in fp16; host
    sums the 8 partials in fp32 (the TP all-reduce, done at unshard time).

Pipeline notes (v4):
  - Superblocks: the two batches' same-index 512-query blocks interleave in
    one j-loop, so the exp stream always has a second independent dependency
    chain to fill stalls and block-boundary count halves.
  - Scores for both heads land in one [128, 1024] PSUM tile (2 banks) written
    by 2 row-packed K=64 matmuls that run concurrently; ONE [128,1024] exp.
  - PV accumulates into a [65, 1024] PSUM tile (ones-row denominator), with a
    2-deep software pipeline lag so PE doesn't wait on exp/mask.
  - Normalization: reciprocal_approx_fast on the SBUF-staged den row (custom
    DVE ops misread PSUM at non-zero bank offsets), broadcast via tiny f32r
    K=1 matmuls, applied by partition-shifted DVE muls (no SBUF-SBUF DMA).
  - QKV+RoPE (phase A) of the next superblock and o_proj of the previous one
    are queue items drip-fed into the current attention loop, keeping all
    engines loaded and the PE clock-gate (HAM) warm.
"""

import os
import numpy as np

# Problem constants (hardcoded per contract; kernel.py must be self-contained).
B = 2
S = 2048
T = B * S          # 4096 flattened tokens
H = 1024           # hidden
HD = 64            # head dim
P = 128
NCORES = 8
THETA = 10000.0
SCALE = HD ** -0.5
NBLK = T // 512    # 8 token blocks of 512
HO = H // P        # 8 hidden chunks of 128
SJT = S // P       # 16 key tiles per batch

_NC_CACHE = {}
LAST_RESULT = None  # stash of the last BassKernelResults (for test harnesses)


def _ensure_ntff_hook():
    """Provide antenv.axon_hooks if the image lacks it, so BASS_TRACE=1
    profiling works under axon instead of crashing on import."""
    import sys
    import types

    try:
        import antenv.axon_hooks  # noqa: F401
        return
    except ImportError:
        pass
    mod = types.ModuleType("antenv.axon_hooks")
    mod._hook = None

    def set_axon_ntff_profile_hook(h):
        mod._hook = h

    def get_axon_ntff_profile_hook():
        return mod._hook

    mod.set_axon_ntff_profile_hook = set_axon_ntff_profile_hook
    mod.get_axon_ntff_profile_hook = get_axon_ntff_profile_hook
    sys.modules["antenv.axon_hooks"] = mod
    try:
        import antenv

        antenv.axon_hooks = mod
    except ImportError:
        pass
    try:
        from trn_agent_boot.trn_boot import _ntff_profile_via_ctypes

        hook = _ntff_profile_via_ctypes("/opt/axon/libaxon_pjrt.so")
        if hook is not None:
            mod.set_axon_ntff_profile_hook(hook)
    except Exception:
        pass


_ensure_ntff_hook()


def _build_nc(mm_mode="bf16"):
    import concourse.bass as bass
    import concourse.mybir as mybir
    import concourse.tile as tile
    from concourse import bacc

    from concourse.dve_ops import RECIPROCAL_APPROX_FAST, RECIP_APPROX_FAST_CONSTS

    f32 = mybir.dt.float32
    f32r = mybir.dt.float32r
    f16 = mybir.dt.float16
    Act = mybir.ActivationFunctionType

    if mm_mode == "f32r":
        adt = mybir.dt.float32r
    elif mm_mode == "f32":
        adt = f32
    elif mm_mode == "bf16":
        adt = mybir.dt.bfloat16
    else:
        raise ValueError(mm_mode)

    nc = bacc.Bacc(None, target_bir_lowering=False, debug=False)

    # --- DRAM I/O ------------------------------------------------------------
    xT_d = nc.dram_tensor("xT", [H, T], adt, kind="ExternalInput")
    wq_d = nc.dram_tensor("wq", [H, P], adt, kind="ExternalInput")
    bq_d = nc.dram_tensor("bq", [P, 1], f32, kind="ExternalInput")
    wkv_d = nc.dram_tensor("wkv", [H, P], adt, kind="ExternalInput")
    bkv_d = nc.dram_tensor("bkv", [P, 1], f32, kind="ExternalInput")
    wo_d = nc.dram_tensor("wo", [P, H], adt, kind="ExternalInput")
    cos_d = nc.dram_tensor("cosT", [P, S], adt, kind="ExternalInput")
    sin_d = nc.dram_tensor("sinT", [P, S], adt, kind="ExternalInput")
    perm_d = nc.dram_tensor("perm", [P, P], adt, kind="ExternalInput")
    id_d = nc.dram_tensor("ident", [P, P], adt, kind="ExternalInput")
    onescol_d = nc.dram_tensor("onescol", [P, 2 * SJT], adt, kind="ExternalInput")
    onesr_d = nc.dram_tensor("onesr", [1, 64], f32r, kind="ExternalInput")
    out_d = nc.dram_tensor("out", [T, H], f16, kind="ExternalOutput")
    debug = os.environ.get("KERNEL_DEBUG") == "1"
    if debug:
        dden_d = nc.dram_tensor("dbg_den", [NBLK, 1024], f32, kind="ExternalOutput")
        dat_d = nc.dram_tensor("dbg_at", [NBLK, P, 512], f32, kind="ExternalOutput")

    with tile.TileContext(nc) as tc:
        with (
            tc.tile_pool(name="const", bufs=1) as cpool,
            tc.tile_pool(name="persist", bufs=1) as ppool,
            tc.tile_pool(name="xt", bufs=3) as xt_pool,
            tc.tile_pool(name="ptile", bufs=5) as p_pool,
            tc.tile_pool(name="attn", bufs=1) as attn_pool,
            tc.tile_pool(name="nrm", bufs=2) as nrm_pool,
            tc.tile_pool(name="tmp", bufs=2) as tmp_pool,
            tc.tile_pool(name="osb", bufs=3) as osb_pool,
            tc.tile_pool(name="ps", bufs=2, space="PSUM") as ps_pool,
        ):
            # --- constant loads ---------------------------------------------
            wq_sb = cpool.tile([P, HO, P], adt)
            nc.sync.dma_start(wq_sb[:], wq_d[:].rearrange("(o p) m -> p o m", p=P))
            wkv_sb = cpool.tile([P, HO, P], adt)
            nc.sync.dma_start(wkv_sb[:], wkv_d[:].rearrange("(o p) m -> p o m", p=P))
            bq_sb = cpool.tile([P, 1], f32)
            nc.sync.dma_start(bq_sb[:], bq_d[:])
            bkv_sb = cpool.tile([P, 1], f32)
            nc.sync.dma_start(bkv_sb[:], bkv_d[:])
            wo_sb = cpool.tile([P, H], adt)
            nc.sync.dma_start(wo_sb[:], wo_d[:])
            cos_sb = cpool.tile([P, S], adt)
            nc.sync.dma_start(cos_sb[:], cos_d[:])
            sin_sb = cpool.tile([P, S], adt)
            nc.sync.dma_start(sin_sb[:], sin_d[:])
            perm_sb = cpool.tile([P, P], adt)
            nc.sync.dma_start(perm_sb[:], perm_d[:])
            id_sb = cpool.tile([P, P], adt)
            nc.sync.dma_start(id_sb[:], id_d[:])
            onesr_sb = cpool.tile([1, 64], f32r)
            nc.sync.dma_start(onesr_sb[:], onesr_d[:])

            # --- persistent activation tiles --------------------------------
            qT = ppool.tile([P, T], adt)        # 2 q heads stacked (rows h*64+d)
            kvT = ppool.tile([P, T], adt)       # k rows 0:64, v rows 64:128
            khi = ppool.tile([P, T], adt)       # k duplicated at rows 64:128
            vnat = ppool.tile([P, 2 * SJT, 72], adt)  # v natural [j, d] + ones col

            # ones column for the PV denominator row (memset can't emit f32r)
            nc.sync.dma_start(
                vnat[:, :, 64:65], onescol_d[:].rearrange("p (j o) -> p j o", o=1)
            )

            xT_r = xT_d[:].rearrange("(o p) t -> p o t", p=P)

            # prefetch first x block
            xts = {}
            xts[0] = xt_pool.tile([P, HO, 512], adt, tag="xt", name="xt0")
            nc.sync.dma_start(xts[0][:], xT_r[:, :, 0:512])

            deferred = []  # queue of (kind, emit_fn) work items

            def pop_deferred(n, on_act=False, norm_only=False):
                for _ in range(n):
                    if not deferred:
                        return
                    if norm_only and deferred[0][0] != "norm":
                        return
                    _, fn = deferred.pop(0)
                    fn(on_act)

            def flush_kind(kind):
                while any(k == kind for k, _ in deferred):
                    _, fn = deferred.pop(
                        next(i for i, (k, _) in enumerate(deferred) if k == kind)
                    )
                    fn(False)

            def phase_a_items(b, ib):
                """QKV projection + RoPE + v-transpose for block (b, ib), as
                queue items drip-fed into the PREVIOUS block's attention loop
                so this work overlaps attention instead of walling between
                j-loops."""
                blk = b * 4 + ib
                tb = slice(blk * 512, (blk + 1) * 512)
                sc = (blk * 512) % S
                ss = slice(sc, sc + 512)
                state = {}

                def item_q1(on_act):
                    xt = xts[blk]
                    qkv_ps = ps_pool.tile([P, 1024], f32, tag="sps", name="qkvps")
                    state["qkv_ps"] = qkv_ps
                    for o in range(4):
                        nc.tensor.matmul(
                            qkv_ps[:, 0:512], wq_sb[:, o, :], xt[:, o, :],
                            start=(o == 0), stop=False,
                        )

                def item_q2(on_act):
                    xt = xts[blk]
                    qkv_ps = state["qkv_ps"]
                    for o in range(4, HO):
                        nc.tensor.matmul(
                            qkv_ps[:, 0:512], wq_sb[:, o, :], xt[:, o, :],
                            start=False, stop=(o == HO - 1),
                        )
                    nc.scalar.activation(
                        qT[:, tb], qkv_ps[:, 0:512], Act.Identity, bias=bq_sb[:]
                    )

                def item_kv1(on_act):
                    xt = xts[blk]
                    qkv_ps = state["qkv_ps"]
                    for o in range(4):
                        nc.tensor.matmul(
                            qkv_ps[:, 512:1024], wkv_sb[:, o, :], xt[:, o, :],
                            start=(o == 0), stop=False,
                        )

                def item_kv2(on_act):
                    xt = xts.pop(blk)
                    qkv_ps = state.pop("qkv_ps")
                    for o in range(4, HO):
                        nc.tensor.matmul(
                            qkv_ps[:, 512:1024], wkv_sb[:, o, :], xt[:, o, :],
                            start=False, stop=(o == HO - 1),
                        )
                    nc.scalar.activation(
                        kvT[:, tb], qkv_ps[:, 512:1024], Act.Identity,
                        bias=bkv_sb[:],
                    )

                def item_ropeA(on_act):
                    pr = ps_pool.tile([P, 1024], f32, tag="sps", name="prps")
                    state["pr"] = pr
                    nc.tensor.matmul(
                        pr[:, 0:512], perm_sb[:], qT[:, tb], start=True, stop=True
                    )
                    nc.tensor.matmul(
                        pr[0:64, 512:1024], perm_sb[0:64, 0:64], kvT[0:64, tb],
                        start=True, stop=True,
                    )
                    tp = ps_pool.tile([P, 4, 64], adt, tag="sps", name="tpps")
                    for jj in range(4):
                        jt = blk * 4 + jj
                        nc.tensor.transpose(
                            tp[:, jj, :], kvT[64:128, jt * P:(jt + 1) * P],
                            id_sb[64:128, 64:128],
                        )
                    # DVE order frees the pr/tp psum slots first (rtmp reads,
                    # vnat copy) so later scores tiles aren't ring-blocked
                    rtmp = tmp_pool.tile([P, 1024], adt, tag="ropetmp")
                    state["rtmp"] = rtmp
                    nc.vector.tensor_mul(
                        rtmp[:, 0:512], pr[:, 0:512], sin_sb[:, ss]
                    )
                    nc.vector.tensor_mul(
                        rtmp[0:64, 512:1024], pr[0:64, 512:1024], sin_sb[0:64, ss]
                    )
                    nc.vector.tensor_copy(
                        vnat[:, blk * 4:(blk + 1) * 4, 0:64], tp[:]
                    )

                def item_ropeB(on_act):
                    rtmp = state.pop("rtmp")
                    state.pop("pr")
                    nc.vector.tensor_mul(qT[:, tb], qT[:, tb], cos_sb[:, ss])
                    nc.vector.tensor_add(qT[:, tb], qT[:, tb], rtmp[:, 0:512])
                    nc.vector.tensor_mul(
                        kvT[0:64, tb], kvT[0:64, tb], cos_sb[0:64, ss]
                    )
                    nc.vector.tensor_add(
                        kvT[0:64, tb], kvT[0:64, tb], rtmp[0:64, 512:1024]
                    )
                    # duplicate rope'd k at rows 64:128 (head-1 scores lhsT)
                    nc.sync.dma_start(khi[64:128, tb], kvT[0:64, tb])

                return [item_q1, item_q2, item_kv1, item_kv2, item_ropeA,
                        item_ropeB]

            def emit_norm_oproj(b, ib, po):
                """normA inline (DVE recip chain); normB + o_proj chunks
                queued for later loops."""
                blk = b * 4 + ib
                # stage den to SBUF: custom-DVE ops misread PSUM at
                # non-zero bank offsets (observed on HW)
                den_sb = nrm_pool.tile([1, 1024], f32, tag="den", name="den")
                nc.vector.tensor_copy(den_sb[:], po[64:65, :])
                # r_row is f32r so the broadcast matmul runs at 1 cyc/row
                # (a plain f32 matmul streams at 4 cyc/col); same bits.
                r_row = nrm_pool.tile([1, 1024], f32r, tag="rrow", name="rrow")
                c = RECIP_APPROX_FAST_CONSTS
                nc.vector._custom_dve(
                    RECIPROCAL_APPROX_FAST, out=r_row[:], in0=den_sb[:],
                    s0=c["s0"], s1=c["s1"], imm2=c["imm2"],
                )
                at = attn_pool.tile([P, 512], adt, tag="at", bufs=5, name="at")

                def normB(on_act, po=po, r_row=r_row, at=at, blk=blk):
                    pb = ps_pool.tile([64, 1024], f32, tag="sps", name="pbps")
                    nc.tensor.matmul(
                        pb[:, 0:512], onesr_sb[:], r_row[:, 0:512],
                        start=True, stop=True,
                    )
                    nc.tensor.matmul(
                        pb[:, 512:1024], onesr_sb[:], r_row[:, 512:1024],
                        start=True, stop=True,
                    )
                    pbs = nrm_pool.tile([64, 1024], adt, tag="pbs", name="pbs")
                    nc.vector.tensor_copy(pbs[:], pb[:])
                    nc.vector.tensor_mul(
                        at[0:64, :], po[0:64, 0:512], pbs[:, 0:512]
                    )
                    # head1 written with a partition-base shift (0:64 ->
                    # 64:128) straight into `at` -- no SBUF-SBUF DMA hop
                    nc.vector.tensor_mul(
                        at[64:128, :], po[0:64, 512:1024], pbs[:, 512:1024]
                    )
                    if debug:
                        dden = nrm_pool.tile([1, 1024], f32, tag="dden",
                                             name="dden")
                        nc.vector.tensor_copy(dden[:], po[64:65, :])
                        nc.sync.dma_start(dden_d[blk:blk + 1, :], dden[:])
                        datt = nrm_pool.tile([P, 512], f32, tag="datt",
                                             name="datt")
                        nc.vector.tensor_copy(datt[:], at[:])
                        nc.sync.dma_start(dat_d[blk, :, :], datt[:])

                deferred.append(("norm", normB))

                for st in range(4):
                    rows = slice(
                        b * S + ib * 512 + st * P, b * S + ib * 512 + (st + 1) * P
                    )

                    def chunk(on_act, at=at, st=st, rows=rows):
                        w_ps = ps_pool.tile([P, 1024], f32, tag="sps",
                                            name="wps")
                        nc.tensor.matmul(
                            w_ps[:, 0:512], at[:, st * P:(st + 1) * P],
                            wo_sb[:, 0:512], start=True, stop=True,
                        )
                        nc.tensor.matmul(
                            w_ps[:, 512:1024], at[:, st * P:(st + 1) * P],
                            wo_sb[:, 512:1024], start=True, stop=True,
                        )
                        osb = osb_pool.tile([P, 1024], f16, tag="osb",
                                            name="osb")
                        if on_act:
                            nc.scalar.activation(osb[:], w_ps[:], Act.Identity)
                        else:
                            nc.vector.tensor_copy(osb[:], w_ps[:])
                        nc.sync.dma_start(out_d[rows, :], osb[:])

                    deferred.append(("chunk", chunk))

            # first superblock's phase A (blocks 0 and 4) runs inline
            for bb in range(B):
                blk0 = bb * 4
                if blk0 not in xts:
                    xts[blk0] = xt_pool.tile(
                        [P, HO, 512], adt, tag="xt", name=f"xt{blk0}"
                    )
                    nc.sync.dma_start(
                        xts[blk0][:], xT_r[:, :, blk0 * 512:(blk0 + 1) * 512]
                    )
                for fn in phase_a_items(bb, 0):
                    fn(False)

            # superblocks: the two batches' (ib) blocks interleave so the exp
            # stream always has a second independent chain to fill stalls
            for ib in range(4):
                if ib + 1 < 4:
                    for bb in range(B):
                        nb = bb * 4 + ib + 1
                        xts[nb] = xt_pool.tile(
                            [P, HO, 512], adt, tag="xt", name=f"xt{nb}"
                        )
                        nc.sync.dma_start(
                            xts[nb][:], xT_r[:, :, nb * 512:(nb + 1) * 512]
                        )
                        for fn in phase_a_items(bb, ib + 1):
                            deferred.append(("phA%d" % (ib + 1), fn))

                flush_kind("phA%d" % ib)

                njt = 4 * (ib + 1)
                pos_ = {}
                ptiles = {}
                for bb in range(B):
                    pos_[bb] = ps_pool.tile(
                        [65, 1024], f32, tag="po", name=f"pops{bb}"
                    )

                def emit_pv(bb, j, pos_=pos_, njt=njt, ptiles=ptiles):
                    jt = bb * SJT + j
                    pj = ptiles.pop((bb, j))
                    nc.tensor.matmul(
                        pos_[bb][:, 0:512], vnat[:, jt, 0:65], pj[:, 0, :],
                        start=(j == 0), stop=(j == njt - 1),
                    )
                    nc.tensor.matmul(
                        pos_[bb][:, 512:1024], vnat[:, jt, 0:65], pj[:, 1, :],
                        start=(j == 0), stop=(j == njt - 1),
                    )

                for j in range(njt):
                    for bb in range(B):
                        jt = bb * SJT + j
                        js = slice(jt * P, (jt + 1) * P)
                        qs = slice(bb * S + ib * 512, bb * S + (ib + 1) * 512)
                        sp = ps_pool.tile([P, 1024], f32, tag="sps", name="spps")
                        nc.tensor.matmul(
                            sp[:, 0:512], kvT[0:64, js], qT[0:64, qs],
                            start=True, stop=True,
                        )
                        nc.tensor.matmul(
                            sp[:, 512:1024], khi[64:128, js], qT[64:128, qs],
                            start=True, stop=True,
                        )
                        pj = p_pool.tile([P, 2, 512], adt, tag="ptile", name="pj")
                        ptiles[(bb, j)] = pj
                        nc.scalar.activation(
                            pj[:].rearrange("p h q -> p (h q)"),
                            sp[:], Act.Exp, scale=SCALE,
                        )
                        if j >= 4 * ib:
                            # causal mask on the diagonal band:
                            # keep when  -p + q + (512*ib - 128*j) >= 0
                            base = 512 * ib - 128 * j
                            nc.gpsimd.affine_select(
                                pj[:], pj[:],
                                pattern=[[0, 2], [1, 512]],
                                compare_op=mybir.AluOpType.is_ge,
                                fill=0.0,
                                base=base,
                                channel_multiplier=-1,
                            )
                    if j >= 2:
                        for bb in range(B):
                            emit_pv(bb, j - 2)
                    if j >= 1:
                        pop_deferred(
                            6 if njt == 4 else (3 if njt == 8 else 2),
                            on_act=(njt <= 8),
                        )
                for jr in range(max(0, njt - 2), njt):
                    for bb in range(B):
                        emit_pv(bb, jr)

                for bb in range(B):
                    emit_norm_oproj(bb, ib, pos_[bb])

            # flush remaining work: norms first, then interleave the two
            # batches' o_proj chunks so independent chains overlap
            norms = [d for d in deferred if d[0] == "norm"]
            chunks = [d for d in deferred if d[0] != "norm"]
            half = (len(chunks) + 1) // 2
            inter = []
            for i in range(half):
                inter.append(chunks[i])
                if half + i < len(chunks):
                    inter.append(chunks[half + i])
            deferred[:] = norms + inter
            pop_deferred(len(deferred))

    nc.compile()
    return nc


def _get_nc(mm_mode="bf16"):
    if mm_mode not in _NC_CACHE:
        _NC_CACHE[mm_mode] = _build_nc(mm_mode)
    return _NC_CACHE[mm_mode]


def make_in_maps(inputs, mm_mode="bf16"):
    """Host-side sharding/layout prep: returns the 8 per-core input dicts."""
    if mm_mode == "bf16":
        import ml_dtypes

        a_np = ml_dtypes.bfloat16
    else:
        a_np = np.float32
    hidden = np.asarray(inputs["hidden_states"], dtype=np.float32)
    pos = np.asarray(inputs["positions"])
    Wq = np.asarray(inputs["Wq"], dtype=np.float32)
    bq = np.asarray(inputs["bq"], dtype=np.float32)
    Wk = np.asarray(inputs["Wk"], dtype=np.float32)
    bk = np.asarray(inputs["bk"], dtype=np.float32)
    Wv = np.asarray(inputs["Wv"], dtype=np.float32)
    bv = np.asarray(inputs["bv"], dtype=np.float32)
    Wo = np.asarray(inputs["Wo"], dtype=np.float32)

    xT = np.ascontiguousarray(hidden.reshape(T, H).T)

    half = HD // 2  # 32
    inv = 1.0 / THETA ** (np.arange(half, dtype=np.float64) * 2.0 / HD)
    f = pos.astype(np.float64)[None, :] * inv[:, None]          # [32, S]
    cos32 = np.cos(f)
    sin32 = np.sin(f)
    pidx = np.arange(P) % half
    sgn = np.where(np.arange(P) % HD < half, -1.0, 1.0)
    cosT = np.ascontiguousarray(cos32[pidx].astype(np.float32))
    sinT = np.ascontiguousarray((sin32[pidx] * sgn[:, None]).astype(np.float32))

    m = np.arange(P)
    sig = np.where(m % HD < half, m + half, m - half)
    perm = np.zeros((P, P), np.float32)
    perm[sig, m] = 1.0
    ident = np.eye(P, dtype=np.float32)
    onescol = np.ones((P, 2 * (S // P)), np.float32)

    xTa = xT.astype(a_np)
    in_maps = []
    for c in range(NCORES):
        g = c // 2  # kv head for this core's 2 q heads
        wkv = np.ascontiguousarray(
            np.concatenate(
                [Wk[:, g * HD:(g + 1) * HD], Wv[:, g * HD:(g + 1) * HD]], axis=1
            )
        )
        bkv = np.ascontiguousarray(
            np.concatenate([bk[g * HD:(g + 1) * HD], bv[g * HD:(g + 1) * HD]])[:, None]
        )
        in_maps.append({
            "xT": xTa,
            "onesr": np.ones((1, 64), np.float32),
            "wq": np.ascontiguousarray(Wq[:, c * P:(c + 1) * P]).astype(a_np),
            "bq": np.ascontiguousarray(bq[c * P:(c + 1) * P][:, None]),
            "wkv": wkv.astype(a_np),
            "bkv": bkv,
            "wo": np.ascontiguousarray(Wo[c * P:(c + 1) * P, :]).astype(a_np),
            "cosT": cosT.astype(a_np),
            "sinT": sinT.astype(a_np),
            "perm": perm.astype(a_np),
            "ident": ident.astype(a_np),
            "onescol": onescol.astype(a_np),
        })
    return in_maps


def kernel(**inputs):
    global LAST_RESULT
    from concourse.bass_utils import run_bass_kernel_spmd

    mm_mode = os.environ.get("KERNEL_MM_MODE", "bf16")
    nc = _get_nc(mm_mode)
    in_maps = make_in_maps(inputs, mm_mode)
    res = run_bass_kernel_spmd(nc, in_maps, core_ids=list(range(NCORES)))
    LAST_RESULT = res
    out = res.results[0]["out"].astype(np.float32, copy=True)
    for rr in res.results[1:]:
        out += rr["out"].astype(np.float32)
    return out.reshape(B, S, H)


# revision 35
# speedup vs baseline: 1.2018x; 1.2018x over previous
"""Trainium2 Bass kernel for MiMoAudio attention (GQA + neox RoPE + causal softmax + o_proj).

Strategy (tensor-parallel over heads, 8 cores):
  - Each core owns 2 of the 16 q heads (128 q channels) and the single kv head
    (64 channels) that those q heads attend to (GQA group).
  - Host pre-transposes hidden_states to xT [H, B*S] so every on-device matmul
    contracts over the partition dim with no on-device transposition of x.
  - All activations live "feature-on-partitions" (transposed domain):
      qT [128, T], kT/vT in kvT [128, T], scoresT [j, i], attnT [d', i].
    Softmax runs without max-subtraction (logits are O(10), fp32-exp safe);
    the denominator is obtained by an appended ones-row in the PV matmul.
  - o_proj row-slice per core produces a partial [T, H] output in fp16; host
    sums the 8 partials in fp32 (the TP all-reduce, done at unshard time).

Pipeline notes (v4):
  - Superblocks: the two batches' same-index 512-query blocks interleave in
    one j-loop, so the exp stream always has a second independent dependency
    chain to fill stalls and block-boundary count halves.
  - Scores for both heads land in one [128, 1024] PSUM tile (2 banks) written
    by 2 row-packed K=64 matmuls that run concurrently; ONE [128,1024] exp.
  - PV accumulates into a [65, 1024] PSUM tile (ones-row denominator), with a
    2-deep software pipeline lag so PE doesn't wait on exp/mask.
  - Normalization: reciprocal_approx_fast on the SBUF-staged den row (custom
    DVE ops misread PSUM at non-zero bank offsets), broadcast via tiny f32r
    K=1 matmuls, applied by partition-shifted DVE muls (no SBUF-SBUF DMA).
  - QKV+RoPE (phase A) of the next superblock and o_proj of the previous one
    are queue items drip-fed into the current attention loop, keeping all
    engines loaded and the PE clock-gate (HAM) warm.
"""

import os
import numpy as np

# Problem constants (hardcoded per contract; kernel.py must be self-contained).
B = 2
S = 2048
T = B * S          # 4096 flattened tokens
H = 1024           # hidden
HD = 64            # head dim
P = 128
NCORES = 8
THETA = 10000.0
SCALE = HD ** -0.5
NBLK = T // 512    # 8 token blocks of 512
HO = H // P        # 8 hidden chunks of 128
SJT = S // P       # 16 key tiles per batch

_NC_CACHE = {}
LAST_RESULT = None  # stash of the last BassKernelResults (for test harnesses)


def _ensure_ntff_hook():
    """Provide antenv.axon_hooks if the image lacks it, so BASS_TRACE=1
    profiling works under axon instead of crashing on import."""
    import sys
    import types

    try:
        import antenv.axon_hooks  # noqa: F401
        return
    except ImportError:
        pass
    mod = types.ModuleType("antenv.axon_hooks")
    mod._hook = None

    def set_axon_ntff_profile_hook(h):
        mod._hook = h

    def get_axon_ntff_profile_hook():
        return mod._hook

    mod.set_axon_ntff_profile_hook = set_axon_ntff_profile_hook
    mod.get_axon_ntff_profile_hook = get_axon_ntff_profile_hook
    sys.modules["antenv.axon_hooks"] = mod
    try:
        import antenv

        antenv.axon_hooks = mod
    except ImportError:
        pass
    try:
        from trn_agent_boot.trn_boot import _ntff_profile_via_ctypes

        hook = _ntff_profile_via_ctypes("/opt/axon/libaxon_pjrt.so")
        if hook is not None:
            mod.set_axon_ntff_profile_hook(hook)
    except Exception:
        pass


_ensure_ntff_hook()


def _build_nc(mm_mode="bf16"):
    import concourse.bass as bass
    import concourse.mybir as mybir
    import concourse.tile as tile
    from concourse import bacc

    from concourse.dve_ops import RECIPROCAL_APPROX_FAST, RECIP_APPROX_FAST_CONSTS

    f32 = mybir.dt.float32
    f32r = mybir.dt.float32r
    f16 = mybir.dt.float16
    Act = mybir.ActivationFunctionType

    if mm_mode == "f32r":
        adt = mybir.dt.float32r
    elif mm_mode == "f32":
        adt = f32
    elif mm_mode == "bf16":
        adt = mybir.dt.bfloat16
    else:
        raise ValueError(mm_mode)

    nc = bacc.Bacc(None, target_bir_lowering=False, debug=False)

    # --- DRAM I/O ------------------------------------------------------------
    xT_d = nc.dram_tensor("xT", [H, T], adt, kind="ExternalInput")
    wq_d = nc.dram_tensor("wq", [H, P], adt, kind="ExternalInput")
    bq_d = nc.dram_tensor("bq", [P, 1], f32, kind="ExternalInput")
    wkv_d = nc.dram_tensor("wkv", [H, P], adt, kind="ExternalInput")
    bkv_d = nc.dram_tensor("bkv", [P, 1], f32, kind="ExternalInput")
    wo_d = nc.dram_tensor("wo", [P, H], adt, kind="ExternalInput")
    cos_d = nc.dram_tensor("cosT", [P, S], adt, kind="ExternalInput")
    sin_d = nc.dram_tensor("sinT", [P, S], adt, kind="ExternalInput")
    perm_d = nc.dram_tensor("perm", [P, P], adt, kind="ExternalInput")
    id_d = nc.dram_tensor("ident", [P, P], adt, kind="ExternalInput")
    onescol_d = nc.dram_tensor("onescol", [P, 2 * SJT], adt, kind="ExternalInput")
    onesr_d = nc.dram_tensor("onesr", [1, 64], f32r, kind="ExternalInput")
    out_d = nc.dram_tensor("out", [T, H], f16, kind="ExternalOutput")
    debug = os.environ.get("KERNEL_DEBUG") == "1"
    if debug:
        dden_d = nc.dram_tensor("dbg_den", [NBLK, 1024], f32, kind="ExternalOutput")
        dat_d = nc.dram_tensor("dbg_at", [NBLK, P, 512], f32, kind="ExternalOutput")

    with tile.TileContext(nc) as tc:
        with (
            tc.tile_pool(name="const", bufs=1) as cpool,
            tc.tile_pool(name="persist", bufs=1) as ppool,
            tc.tile_pool(name="xt", bufs=3) as xt_pool,
            tc.tile_pool(name="ptile", bufs=5) as p_pool,
            tc.tile_pool(name="attn", bufs=1) as attn_pool,
            tc.tile_pool(name="nrm", bufs=2) as nrm_pool,
            tc.tile_pool(name="tmp", bufs=2) as tmp_pool,
            tc.tile_pool(name="osb", bufs=3) as osb_pool,
            tc.tile_pool(name="ps", bufs=2, space="PSUM") as ps_pool,
        ):
            # --- constant loads ---------------------------------------------
            wq_sb = cpool.tile([P, HO, P], adt)
            nc.sync.dma_start(wq_sb[:], wq_d[:].rearrange("(o p) m -> p o m", p=P))
            wkv_sb = cpool.tile([P, HO, P], adt)
            nc.sync.dma_start(wkv_sb[:], wkv_d[:].rearrange("(o p) m -> p o m", p=P))
            bq_sb = cpool.tile([P, 1], f32)
            nc.sync.dma_start(bq_sb[:], bq_d[:])
            bkv_sb = cpool.tile([P, 1], f32)
            nc.sync.dma_start(bkv_sb[:], bkv_d[:])
            wo_sb = cpool.tile([P, H], adt)
            nc.sync.dma_start(wo_sb[:], wo_d[:])
            cos_sb = cpool.tile([P, S], adt)
            nc.sync.dma_start(cos_sb[:], cos_d[:])
            sin_sb = cpool.tile([P, S], adt)
            nc.sync.dma_start(sin_sb[:], sin_d[:])
            perm_sb = cpool.tile([P, P], adt)
            nc.sync.dma_start(perm_sb[:], perm_d[:])
            id_sb = cpool.tile([P, P], adt)
            nc.sync.dma_start(id_sb[:], id_d[:])
            onesr_sb = cpool.tile([1, 64], f32r)
            nc.sync.dma_start(onesr_sb[:], onesr_d[:])

            # --- persistent activation tiles --------------------------------
            qT = ppool.tile([P, T], adt)        # 2 q heads stacked (rows h*64+d)
            kvT = ppool.tile([P, T], adt)       # k rows 0:64, v rows 64:128
            khi = ppool.tile([P, T], adt)       # k duplicated at rows 64:128
            vnat = ppool.tile([P, 2 * SJT, 72], adt)  # v natural [j, d] + ones col

            # ones column for the PV denominator row (memset can't emit f32r)
            nc.sync.dma_start(
                vnat[:, :, 64:65], onescol_d[:].rearrange("p (j o) -> p j o", o=1)
            )

            xT_r = xT_d[:].rearrange("(o p) t -> p o t", p=P)

            # prefetch first x block
            xts = {}
            xts[0] = xt_pool.tile([P, HO, 512], adt, tag="xt", name="xt0")
            nc.sync.dma_start(xts[0][:], xT_r[:, :, 0:512])

            deferred = []  # queue of (kind, emit_fn) work items

            def pop_deferred(n, on_act=False, norm_only=False):
                for _ in range(n):
                    if not deferred:
                        return
                    if norm_only and deferred[0][0] != "norm":
                        return
                    _, fn = deferred.pop(0)
                    fn(on_act)

            def flush_kind(kind):
                while any(k == kind for k, _ in deferred):
                    _, fn = deferred.pop(
                        next(i for i, (k, _) in enumerate(deferred) if k == kind)
                    )
                    fn(False)

            def phase_a_items(b, ib):
                """QKV projection + RoPE + v-transpose for block (b, ib), as
                queue items drip-fed into the PREVIOUS block's attention loop
                so this work overlaps attention instead of walling between
                j-loops."""
                blk = b * 4 + ib
                tb = slice(blk * 512, (blk + 1) * 512)
                sc = (blk * 512) % S
                ss = slice(sc, sc + 512)
                state = {}

                def item_q(on_act):
                    xt = xts[blk]
                    qkv_ps = ps_pool.tile([P, 1024], f32, tag="sps", name="qkvps")
                    state["qkv_ps"] = qkv_ps
                    for o in range(HO):
                        nc.tensor.matmul(
                            qkv_ps[:, 0:512], wq_sb[:, o, :], xt[:, o, :],
                            start=(o == 0), stop=(o == HO - 1),
                        )
                    nc.scalar.activation(
                        qT[:, tb], qkv_ps[:, 0:512], Act.Identity, bias=bq_sb[:]
                    )

                def item_kv(on_act):
                    xt = xts.pop(blk)
                    qkv_ps = state.pop("qkv_ps")
                    for o in range(HO):
                        nc.tensor.matmul(
                            qkv_ps[:, 512:1024], wkv_sb[:, o, :], xt[:, o, :],
                            start=(o == 0), stop=(o == HO - 1),
                        )
                    nc.scalar.activation(
                        kvT[:, tb], qkv_ps[:, 512:1024], Act.Identity,
                        bias=bkv_sb[:],
                    )

                def item_rope(on_act):
                    pr = ps_pool.tile([P, 1024], f32, tag="sps", name="prps")
                    nc.tensor.matmul(
                        pr[:, 0:512], perm_sb[:], qT[:, tb], start=True, stop=True
                    )
                    nc.tensor.matmul(
                        pr[0:64, 512:1024], perm_sb[0:64, 0:64], kvT[0:64, tb],
                        start=True, stop=True,
                    )
                    tp = ps_pool.tile([P, 4, 64], adt, tag="sps", name="tpps")
                    for jj in range(4):
                        jt = blk * 4 + jj
                        nc.tensor.transpose(
                            tp[:, jj, :], kvT[64:128, jt * P:(jt + 1) * P],
                            id_sb[64:128, 64:128],
                        )
                    # DVE order frees the pr/tp psum slots first (rtmp reads,
                    # vnat copy) so later scores tiles aren't ring-blocked
                    rtmp = tmp_pool.tile([P, 1024], adt, tag="ropetmp")
                    nc.vector.tensor_mul(
                        rtmp[:, 0:512], pr[:, 0:512], sin_sb[:, ss]
                    )
                    nc.vector.tensor_mul(
                        rtmp[0:64, 512:1024], pr[0:64, 512:1024], sin_sb[0:64, ss]
                    )
                    nc.vector.tensor_copy(
                        vnat[:, blk * 4:(blk + 1) * 4, 0:64], tp[:]
                    )
                    nc.vector.tensor_mul(qT[:, tb], qT[:, tb], cos_sb[:, ss])
                    nc.vector.tensor_add(qT[:, tb], qT[:, tb], rtmp[:, 0:512])
                    nc.vector.tensor_mul(
                        kvT[0:64, tb], kvT[0:64, tb], cos_sb[0:64, ss]
                    )
                    nc.vector.tensor_add(
                        kvT[0:64, tb], kvT[0:64, tb], rtmp[0:64, 512:1024]
                    )
                    # duplicate rope'd k at rows 64:128 (head-1 scores lhsT)
                    nc.sync.dma_start(khi[64:128, tb], kvT[0:64, tb])

                return [item_q, item_kv, item_rope]

            def emit_norm_oproj(b, ib, po):
                """normA inline (DVE recip chain); normB + o_proj chunks
                queued for later loops."""
                blk = b * 4 + ib
                # stage den to SBUF: custom-DVE ops misread PSUM at
                # non-zero bank offsets (observed on HW)
                den_sb = nrm_pool.tile([1, 1024], f32, tag="den", name="den")
                nc.vector.tensor_copy(den_sb[:], po[64:65, :])
                # r_row is f32r so the broadcast matmul runs at 1 cyc/row
                # (a plain f32 matmul streams at 4 cyc/col); same bits.
                r_row = nrm_pool.tile([1, 1024], f32r, tag="rrow", name="rrow")
                c = RECIP_APPROX_FAST_CONSTS
                nc.vector._custom_dve(
                    RECIPROCAL_APPROX_FAST, out=r_row[:], in0=den_sb[:],
                    s0=c["s0"], s1=c["s1"], imm2=c["imm2"],
                )
                at = attn_pool.tile([P, 512], adt, tag="at", bufs=5, name="at")

                def normB(on_act, po=po, r_row=r_row, at=at, blk=blk):
                    pb = ps_pool.tile([64, 1024], f32, tag="sps", name="pbps")
                    nc.tensor.matmul(
                        pb[:, 0:512], onesr_sb[:], r_row[:, 0:512],
                        start=True, stop=True,
                    )
                    nc.tensor.matmul(
                        pb[:, 512:1024], onesr_sb[:], r_row[:, 512:1024],
                        start=True, stop=True,
                    )
                    pbs = nrm_pool.tile([64, 1024], adt, tag="pbs", name="pbs")
                    nc.vector.tensor_copy(pbs[:], pb[:])
                    nc.vector.tensor_mul(
                        at[0:64, :], po[0:64, 0:512], pbs[:, 0:512]
                    )
                    # head1 written with a partition-base shift (0:64 ->
                    # 64:128) straight into `at` -- no SBUF-SBUF DMA hop
                    nc.vector.tensor_mul(
                        at[64:128, :], po[0:64, 512:1024], pbs[:, 512:1024]
                    )
                    if debug:
                        dden = nrm_pool.tile([1, 1024], f32, tag="dden",
                                             name="dden")
                        nc.vector.tensor_copy(dden[:], po[64:65, :])
                        nc.sync.dma_start(dden_d[blk:blk + 1, :], dden[:])
                        datt = nrm_pool.tile([P, 512], f32, tag="datt",
                                             name="datt")
                        nc.vector.tensor_copy(datt[:], at[:])
                        nc.sync.dma_start(dat_d[blk, :, :], datt[:])

                deferred.append(("norm", normB))

                for st in range(4):
                    rows = slice(
                        b * S + ib * 512 + st * P, b * S + ib * 512 + (st + 1) * P
                    )

                    def chunk(on_act, at=at, st=st, rows=rows):
                        w_ps = ps_pool.tile([P, 1024], f32, tag="sps",
                                            name="wps")
                        nc.tensor.matmul(
                            w_ps[:, 0:512], at[:, st * P:(st + 1) * P],
                            wo_sb[:, 0:512], start=True, stop=True,
                        )
                        nc.tensor.matmul(
                            w_ps[:, 512:1024], at[:, st * P:(st + 1) * P],
                            wo_sb[:, 512:1024], start=True, stop=True,
                        )
                        osb = osb_pool.tile([P, 1024], f16, tag="osb",
                                            name="osb")
                        if on_act:
                            nc.scalar.activation(osb[:], w_ps[:], Act.Identity)
                        else:
                            nc.vector.tensor_copy(osb[:], w_ps[:])
                        nc.sync.dma_start(out_d[rows, :], osb[:])

                    deferred.append(("chunk", chunk))

            # first superblock's phase A (blocks 0 and 4) runs inline
            for bb in range(B):
                blk0 = bb * 4
                if blk0 not in xts:
                    xts[blk0] = xt_pool.tile(
                        [P, HO, 512], adt, tag="xt", name=f"xt{blk0}"
                    )
                    nc.sync.dma_start(
                        xts[blk0][:], xT_r[:, :, blk0 * 512:(blk0 + 1) * 512]
                    )
                for fn in phase_a_items(bb, 0):
                    fn(False)

            # superblocks: the two batches' (ib) blocks interleave so the exp
            # stream always has a second independent chain to fill stalls
            for ib in range(4):
                if ib + 1 < 4:
                    for bb in range(B):
                        nb = bb * 4 + ib + 1
                        xts[nb] = xt_pool.tile(
                            [P, HO, 512], adt, tag="xt", name=f"xt{nb}"
                        )
                        nc.sync.dma_start(
                            xts[nb][:], xT_r[:, :, nb * 512:(nb + 1) * 512]
                        )
                        for fn in phase_a_items(bb, ib + 1):
                            deferred.append(("phA%d" % (ib + 1), fn))

                flush_kind("phA%d" % ib)

                njt = 4 * (ib + 1)
                pos_ = {}
                ptiles = {}
                for bb in range(B):
                    pos_[bb] = ps_pool.tile(
                        [65, 1024], f32, tag="po", name=f"pops{bb}"
                    )

                def emit_pv(bb, j, pos_=pos_, njt=njt, ptiles=ptiles):
                    jt = bb * SJT + j
                    pj = ptiles.pop((bb, j))
                    nc.tensor.matmul(
                        pos_[bb][:, 0:512], vnat[:, jt, 0:65], pj[:, 0, :],
                        start=(j == 0), stop=(j == njt - 1),
                    )
                    nc.tensor.matmul(
                        pos_[bb][:, 512:1024], vnat[:, jt, 0:65], pj[:, 1, :],
                        start=(j == 0), stop=(j == njt - 1),
                    )

                for j in range(njt):
                    for bb in range(B):
                        jt = bb * SJT + j
                        js = slice(jt * P, (jt + 1) * P)
                        qs = slice(bb * S + ib * 512, bb * S + (ib + 1) * 512)
                        sp = ps_pool.tile([P, 1024], f32, tag="sps", name="spps")
                        nc.tensor.matmul(
                            sp[:, 0:512], kvT[0:64, js], qT[0:64, qs],
                            start=True, stop=True,
                        )
                        nc.tensor.matmul(
                            sp[:, 512:1024], khi[64:128, js], qT[64:128, qs],
                            start=True, stop=True,
                        )
                        pj = p_pool.tile([P, 2, 512], adt, tag="ptile", name="pj")
                        ptiles[(bb, j)] = pj
                        nc.scalar.activation(
                            pj[:].rearrange("p h q -> p (h q)"),
                            sp[:], Act.Exp, scale=SCALE,
                        )
                        if j >= 4 * ib:
                            # causal mask on the diagonal band:
                            # keep when  -p + q + (512*ib - 128*j) >= 0
                            base = 512 * ib - 128 * j
                            nc.gpsimd.affine_select(
                                pj[:], pj[:],
                                pattern=[[0, 2], [1, 512]],
                                compare_op=mybir.AluOpType.is_ge,
                                fill=0.0,
                                base=base,
                                channel_multiplier=-1,
                            )
                    if j >= 2:
                        for bb in range(B):
                            emit_pv(bb, j - 2)
                    if j >= 1:
                        pop_deferred(
                            5 if njt == 4 else (3 if njt == 8 else 2),
                            on_act=(njt <= 8),
                        )
                for jr in range(max(0, njt - 2), njt):
                    for bb in range(B):
                        emit_pv(bb, jr)

                for bb in range(B):
                    emit_norm_oproj(bb, ib, pos_[bb])

            # flush remaining work: norms first, then interleave the two
            # batches' o_proj chunks so independent chains overlap
            norms = [d for d in deferred if d[0] == "norm"]
            chunks = [d for d in deferred if d[0] != "norm"]
            half = (len(chunks) + 1) // 2
            inter = []
            for i in range(half):
                inter.append(chunks[i])
                if half + i < len(chunks):
                    inter.append(chunks[half + i])
            deferred[:] = norms + inter
            pop_deferred(len(deferred))

    nc.compile()
    return nc


def _get_nc(mm_mode="bf16"):
    if mm_mode not in _NC_CACHE:
        _NC_CACHE[mm_mode] = _build_nc(mm_mode)
    return _NC_CACHE[mm_mode]


def make_in_maps(inputs, mm_mode="bf16"):
    """Host-side sharding/layout prep: returns the 8 per-core input dicts."""
    if mm_mode == "bf16":
        import ml_dtypes

        a_np = ml_dtypes.bfloat16
    else:
        a_np = np.float32
    hidden = np.asarray(inputs["hidden_states"], dtype=np.float32)
    pos = np.asarray(inputs["positions"])
    Wq = np.asarray(inputs["Wq"], dtype=np.float32)
    bq = np.asarray(inputs["bq"], dtype=np.float32)
    Wk = np.asarray(inputs["Wk"], dtype=np.float32)
    bk = np.asarray(inputs["bk"], dtype=np.float32)
    Wv = np.asarray(inputs["Wv"], dtype=np.float32)
    bv = np.asarray(inputs["bv"], dtype=np.float32)
    Wo = np.asarray(inputs["Wo"], dtype=np.float32)

    xT = np.ascontiguousarray(hidden.reshape(T, H).T)

    half = HD // 2  # 32
    inv = 1.0 / THETA ** (np.arange(half, dtype=np.float64) * 2.0 / HD)
    f = pos.astype(np.float64)[None, :] * inv[:, None]          # [32, S]
    cos32 = np.cos(f)
    sin32 = np.sin(f)
    pidx = np.arange(P) % half
    sgn = np.where(np.arange(P) % HD < half, -1.0, 1.0)
    cosT = np.ascontiguousarray(cos32[pidx].astype(np.float32))
    sinT = np.ascontiguousarray((sin32[pidx] * sgn[:, None]).astype(np.float32))

    m = np.arange(P)
    sig = np.where(m % HD < half, m + half, m - half)
    perm = np.zeros((P, P), np.float32)
    perm[sig, m] = 1.0
    ident = np.eye(P, dtype=np.float32)
    onescol = np.ones((P, 2 * (S // P)), np.float32)

    xTa = xT.astype(a_np)
    in_maps = []
    for c in range(NCORES):
        g = c // 2  # kv head for this core's 2 q heads
        wkv = np.ascontiguousarray(
            np.concatenate(
                [Wk[:, g * HD:(g + 1) * HD], Wv[:, g * HD:(g + 1) * HD]], axis=1
            )
        )
        bkv = np.ascontiguousarray(
            np.concatenate([bk[g * HD:(g + 1) * HD], bv[g * HD:(g + 1) * HD]])[:, None]
        )
        in_maps.append({
            "xT": xTa,
            "onesr": np.ones((1, 64), np.float32),
            "wq": np.ascontiguousarray(Wq[:, c * P:(c + 1) * P]).astype(a_np),
            "bq": np.ascontiguousarray(bq[c * P:(c + 1) * P][:, None]),
            "wkv": wkv.astype(a_np),
            "bkv": bkv,
            "wo": np.ascontiguousarray(Wo[c * P:(c + 1) * P, :]).astype(a_np),
            "cosT": cosT.astype(a_np),
            "sinT": sinT.astype(a_np),
            "perm": perm.astype(a_np),
            "ident": ident.astype(a_np),
            "onescol": onescol.astype(a_np),
        })
    return in_maps


def kernel(**inputs):
    global LAST_RESULT
    from concourse.bass_utils import run_bass_kernel_spmd

    mm_mode = os.environ.get("KERNEL_MM_MODE", "bf16")
    nc = _get_nc(mm_mode)
    in_maps = make_in_maps(inputs, mm_mode)
    res = run_bass_kernel_spmd(nc, in_maps, core_ids=list(range(NCORES)))
    LAST_RESULT = res
    out = res.results[0]["out"].astype(np.float32, copy=True)
    for rr in res.results[1:]:
        out += rr["out"].astype(np.float32)
    return out.reshape(B, S, H)
